# revision 21
# baseline (speedup 1.0000x reference)
"""Trainium2 Bass kernel for DifferentiableSparseHypergraph (topk_masking).

Full computation per batch n:
  x_mean = x[n].mean(T)                      (C, V)
  q = Wq @ x_mean + bq                       (O=32, V)   [1x1 conv == matmul]
  q = q / max(||q||_2 over O, eps)
  H_raw = (q^T @ key_prototypes) / sqrt(O)   (V, M=128)
  topk10 -> softmax over the 10 vals -> scatter back; zeros elsewhere.

Kernel strategy (pure data-parallel over batch, 8 cores x 8 batches).
The kernel is DVE-bound (the t-mean tree must touch every element once
at ~1.1 ns/col; fp32 matmuls on the PE cost ~2.4 ns/col as LOW/HIGH
pairs, so offloading reduction to the PE loses), so everything is
arranged to keep the in-order DVE queue stall-free and its total work
minimal:
  * batches 0..6: one 4 MiB DMA per batch ([c, (half, t, v)]); t-mean
    via a 3-level DVE add tree per c-half (few BIG instructions -- DVE
    per-instruction overhead is ~100 ns, so many small ops lose ~20%
    throughput), a PSUM-accumulated N=512 fp32 matmul per half, and a
    strided-PSUM reduce over the 8 surviving t-groups.
  * each batch's reduce (which needs the PE matmul result) is emitted
    in the MIDDLE of the next batch's tree, so the in-order DVE queue
    never idles waiting for the PE (~1.2 us/batch otherwise).
  * batch 7 (LAST) streams in 1 MiB chunks with chunk-local 4-level
    trees and one N=256 matmul per c-half, so the compute tail after
    the final chunk lands is small instead of a whole batch's serial
    chain (~14 us measured on the coarse variant).
  * the score pipeline is split in two emission points so the DVE queue
    never stalls on cross-engine latency: the ACT/PE part (qsq, pb, pc,
    rn, e) is emitted right after the pair's q2; the DVE part (top-8
    threshold chain + masked-softmax) one batch later, by which time
    pb/e are long since ready. The last pair is emitted fused, with the
    DVE top-8 chain right after the pb matmul and pb before pc so the
    two engine chains overlap.
  * top-10 per row is index-free and SCALE-INVARIANT: the threshold
    t_k (10th largest) is found on the raw scores pb = q2^T kp; the
    normalization rn = (INTER*||q||^2)^-1/2 is applied inside the exp
    as a per-partition ACT scale. rn itself is exp(-0.5*ln(INTER*pc)):
    ln+exp+identity+copy+square all live in ONE activation table
    (natural_log_exp_and_others), and insert_act_table_loads is pinned
    to that table, so the kernel does zero mid-stream ACT table reloads
    (the stock greedy chooser flip-flops ln->natural_log /
    exp->exp_and_others at 1.28 us per swap, 2 per pair).
  * out = (pb >= t_k) * exp(rn*pb) / sum(...) == softmax over the top-10
    scattered back (softmax is shift/subset invariant).
"""

import types

import numpy as np

import concourse.bacc as bacc
import concourse.bass as bass
import concourse.mybir as mybir
import concourse.tile as tile

N, C, T, V = 64, 256, 64, 64
INTER = 32          # conv out channels
M = 128             # num hyperedges
TOPK = 10
NCORES = 8
FP = mybir.dt.float32
NEG_BIG = -1.0e30
TQ = 4              # t-quarters per c-half (1 MiB DMA chunks, last batch)
TC = T // TQ        # t's per chunk (16)
CHUNK_F = TC * V    # chunk free size (1024)
HF = T * V          # per-half free size (4096)

PINNED_ACT_TABLE = "natural_log_exp_and_others"


def _pinned_act_table_loads(self):
    """Replacement for Bacc.insert_act_table_loads on THIS instance only:
    mask every activation-function set except the pinned one so the
    fixpoint chooser can't flip-flop between tables (every function this
    kernel uses -- identity/copy/square/ln/exp -- lives in the pinned
    set). Set ids keep their positions so walrus still maps to the right
    runtime table."""
    import bass_rust as _bass_rust
    from concourse.hw_specs import get_activation_tables

    has_activation = any(
        isinstance(i, mybir.InstActivation)
        for b in self.main_func.blocks
        for i in b.instructions
    )
    if not has_activation:
        return
    tables = list(get_activation_tables(self.m.arch).items())
    assert any(name == PINNED_ACT_TABLE for name, _ in tables), tables
    pinned = [
        (name, (fns if name == PINNED_ACT_TABLE else set()))
        for name, fns in tables
    ]
    _bass_rust.insert_act_table_loads(self, pinned)


def build_nc(nloc: int) -> bass.Bass:
    """Build the per-core Bass program processing `nloc` batches."""
    assert nloc % 2 == 0
    # Bacc (not bare Bass): its compile()/finalize() pipeline splits
    # multi-semaphore waits into InstEventSemaphore pairs — walrus allows
    # at most one sync wait per regular instruction.
    nc = bacc.Bacc(target_bir_lowering=False, debug=False)
    nc.insert_act_table_loads = types.MethodType(_pinned_act_table_loads, nc)

    x = nc.dram_tensor("x", (nloc, C, T, V), FP, kind="ExternalInput")
    wqt = nc.dram_tensor("wqt", (C, INTER), FP, kind="ExternalInput")
    kp = nc.dram_tensor("kp", (INTER, M), FP, kind="ExternalInput")
    bq = nc.dram_tensor("bq", (INTER, 1), FP, kind="ExternalInput")
    out = nc.dram_tensor("out", (nloc, V, M), FP, kind="ExternalOutput")

    A = mybir.AluOpType
    AF = mybir.ActivationFunctionType
    from concourse.tile import add_dep_helper

    with tile.TileContext(nc) as tc:
        with (
            tc.tile_pool(name="consts", bufs=1) as consts,
            tc.tile_pool(name="xph", bufs=4) as xph,
            tc.tile_pool(name="xcp", bufs=8) as xcp,
            tc.tile_pool(name="xcs", bufs=2) as xcs,
            tc.tile_pool(name="xp", bufs=1) as xp,
            tc.tile_pool(name="small", bufs=2) as small,
            tc.tile_pool(name="psA", bufs=2, space="PSUM") as psA,
            tc.tile_pool(name="psB", bufs=2, space="PSUM") as psB,
            tc.tile_pool(name="psS", bufs=1, space="PSUM") as psS,
        ):
            # --- batch 0's x loads go FIRST (before the const DMAs) and in
            # 1 MiB t-range chunks, so the DVE tree starts as early as
            # possible.  Batch 0's first level pairs t with t+1 (each chunk
            # is self-contained); later batches pair t with t+32.
            xh0 = [
                xph.tile([128, HF], FP, tag=f"xh{h}", name=f"xh0_{h}")
                for h in range(2)
            ]
            for h in range(2):
                eng = nc.sync if h == 0 else nc.scalar
                for c in range(2):
                    eng.dma_start(
                        out=xh0[h][:, c * 2048 : (c + 1) * 2048],
                        in_=x[0, h * 128 : (h + 1) * 128,
                              c * (T // 2) : (c + 1) * (T // 2)],
                    )

            # --- replicated constants ---
            wq_sb = consts.tile([128, 2, INTER], FP)    # [c, c_half, o]
            nc.sync.dma_start(
                out=wq_sb[:], in_=wqt.rearrange("(h c) o -> c h o", h=2)
            )
            kp_sb = consts.tile([INTER, M], FP)
            nc.sync.dma_start(out=kp_sb[:], in_=kp[:])
            bq_sb = consts.tile([INTER, 1], FP)
            nc.sync.dma_start(out=bq_sb[:], in_=bq[:])
            ones_sb = consts.tile([INTER, 1], FP)
            nc.vector.memset(ones_sb[:], 1.0)

            # The fp32 self-loading matmul can carry at most ONE semaphore
            # wait (walrus S3_LW_STRUCT limit). Absorb the wq/kp DMA waits
            # with dummy 1x1 matmuls so the first real matmuls only wait on
            # their x-tile DMA.
            scr = psS.tile([1, 1], FP)
            d1 = nc.tensor.matmul(
                scr[:], wq_sb[:, 0, 0:1], wq_sb[:, 0, 0:1], start=True, stop=True
            )
            d2 = nc.tensor.matmul(
                scr[:], kp_sb[:, 0:1], kp_sb[:, 0:1], start=True, stop=True
            )
            add_dep_helper(d2.ins, d1.ins, sync=False, reason="pe-wait-absorb order")

            def emit_pb_chain(q2t, fused_tail_p=None):
                """ACT/PE part of a pair's scores: raw scores pb, row
                norms, e = exp(rn*pb). With fused_tail_p set (last pair),
                the DVE top-8 chain is emitted right after the pb matmul
                so both engine chains run concurrently in the tail."""
                qsq = small.tile([INTER, 2 * V], FP, tag="qsq")
                nc.scalar.activation(qsq[:], q2t[:], AF.Square)
                pb = psB.tile([2 * V, M], FP, tag="pb")
                nc.tensor.matmul(pb[:], q2t[:], kp_sb[:], start=True, stop=True)
                pc = psB.tile([2 * V, 1], FP, tag="pc")
                nc.tensor.matmul(
                    pc[:], qsq[:], ones_sb[:], start=True, stop=True
                )
                thr = emit_thresh(pb) if fused_tail_p is not None else None
                # rn = (INTER*||q||^2)^-0.5 = exp(-0.5*ln(INTER*pc))
                lns = small.tile([2 * V, 1], FP, tag="lns")
                nc.scalar.activation(lns[:], pc[:], AF.Ln, scale=float(INTER))
                rn = small.tile([2 * V, 1], FP, tag="rn")
                nc.scalar.activation(rn[:], lns[:], AF.Exp, scale=-0.5)
                e = small.tile([2 * V, M], FP, tag="e")
                nc.scalar.activation(e[:], pb[:], AF.Exp, scale=rn[:])
                if fused_tail_p is not None:
                    emit_softmax_tail(fused_tail_p, pb, e, thr)
                return pb, e

            def emit_thresh(pb):
                """DVE top-8 chain: t_k = 10th largest raw score per row
                (top8, knock them out, top8 again -> [:,1])."""
                top8a = small.tile([2 * V, 8], FP, tag="t8a")
                nc.vector.max(top8a[:], pb[:])
                work = small.tile([2 * V, M], FP, tag="work")
                nc.vector.match_replace(work[:], top8a[:], pb[:], NEG_BIG)
                top8b = small.tile([2 * V, 8], FP, tag="t8b")
                nc.vector.max(top8b[:], work[:])
                return top8b

            def emit_softmax_tail(p, pb, e, top8b):
                """masked softmax without scatter:
                me = (pb >= t_k) * e; out = me / sum(me)"""
                me = small.tile([2 * V, M], FP, tag="me")
                s = small.tile([2 * V, 1], FP, tag="s")
                nc.vector.scalar_tensor_tensor(
                    out=me[:],
                    in0=pb[:],
                    scalar=top8b[:, 1:2],
                    in1=e[:],
                    op0=A.is_ge,
                    op1=A.mult,
                    accum_out=s[:],
                )
                r = small.tile([2 * V, 1], FP, tag="r")
                nc.vector.reciprocal(r[:], s[:])
                ot = small.tile([2 * V, M], FP, tag="ot")
                nc.scalar.activation(ot[:], me[:], AF.Copy, scale=r[:])
                # out DMA on the ACT HWDGE ring so it never queues in front
                # of x-stream packets on the Sync ring's queue.
                nc.scalar.dma_start(
                    out=out[2 * p : 2 * p + 2].rearrange("b v m -> (b v) m"),
                    in_=ot[:],
                )

            def emit_score_tail(p, pb, e):
                emit_softmax_tail(p, pb, e, emit_thresh(pb))

            q2 = None
            first_mm = None
            pending_tail = None     # (p, pb, e) DVE tail, delayed 1 batch
            pending_q = None        # (pq_psum, n, groups), delayed 1/2 batch

            def flush_pending():
                """Emit the previous batch's reduce+q2 (its PE matmul has
                had a whole tree's worth of time to finish), then the
                1-batch-old DVE score tail, then — if the flushed batch
                completed a pair — that pair's ACT/PE score chain."""
                nonlocal pending_q, pending_tail, q2
                if pending_q is None:
                    return
                pqd, nd, ngroups = pending_q
                pending_q = None
                ld = nd % 2
                qtmp = small.tile([INTER, V], FP, tag="qtmp")
                nc.vector.reduce_sum(
                    out=qtmp[:],
                    in_=pqd[:, : ngroups * V].rearrange(
                        "o (t v) -> o v t", t=ngroups
                    ),
                    axis=mybir.AxisListType.X,
                )
                # q = qsum/T + bq on the idle ACT engine
                if ld == 0:
                    q2 = small.tile([INTER, 2 * V], FP, tag="q2")
                nc.scalar.activation(
                    q2[:, ld * V : (ld + 1) * V],
                    qtmp[:],
                    AF.Identity,
                    bias=bq_sb[:],
                    scale=1.0 / T,
                )
                if pending_tail is not None:
                    emit_score_tail(*pending_tail)
                    pending_tail = None
                if ld == 1:
                    pb, e = emit_pb_chain(q2)
                    pending_tail = (nd // 2, pb, e)

            for n in range(nloc):
                l = n % 2
                last = n == nloc - 1

                if last:
                    # --- fine-grained last batch: 1 MiB chunks, chunk-local
                    # 4-level trees, one N=256 matmul per c-half. (Same pool
                    # tag/shape as the coarse path so PSUM stays in budget.)
                    pqf = psA.tile([INTER, 512], FP, tag="pq")
                    for h in range(2):
                        cs7 = xcs.tile([128, TQ * V], FP, tag=f"cs7{h}")
                        for cq in range(TQ):
                            xc = xcp.tile([128, CHUNK_F], FP, tag="xc")
                            (nc.sync if h == 0 else nc.scalar).dma_start(
                                out=xc[:],
                                in_=x[n, h * 128 : (h + 1) * 128,
                                      cq * TC : (cq + 1) * TC],
                            )
                            c1 = xcs.tile([128, CHUNK_F // 2], FP, tag="c1")
                            nc.vector.tensor_add(
                                c1[:], xc[:, : CHUNK_F // 2],
                                xc[:, CHUNK_F // 2 :],
                            )
                            # 1x1 warm-up matmul gated on this chunk's c1 so
                            # the PE sees activity every ~2 us through the
                            # last batch: HAM otherwise drops to K=4/8 after
                            # ~3 us idle and the tail matmuls (mm7, pb, pc)
                            # run at HALF rate (measured 2.5 ns/col vs 1.2).
                            nc.tensor.matmul(
                                scr[:], c1[0:1, 0:1], c1[0:1, 0:1],
                                start=True, stop=True,
                            )
                            if h == 0 and cq == 1:
                                flush_pending()
                            c2 = xcs.tile([128, CHUNK_F // 4], FP, tag="c2")
                            nc.vector.tensor_add(
                                c2[:], c1[:, : CHUNK_F // 4],
                                c1[:, CHUNK_F // 4 :],
                            )
                            c3 = xcs.tile([128, CHUNK_F // 8], FP, tag="c3")
                            nc.vector.tensor_add(
                                c3[:], c2[:, : CHUNK_F // 8],
                                c2[:, CHUNK_F // 8 :],
                            )
                            nc.vector.tensor_add(
                                cs7[:, cq * V : (cq + 1) * V],
                                c3[:, :V], c3[:, V:],
                            )
                        nc.tensor.matmul(
                            pqf[:, : TQ * V], wq_sb[:, h, :], cs7[:],
                            start=(h == 0), stop=(h == 1),
                        )
                    pending_q = (pqf, n, TQ)
                else:
                    if n == 0:
                        xh = xh0
                    else:
                        xh = []
                        for h in range(2):
                            t = xph.tile([128, HF], FP, tag=f"xh{h}")
                            (nc.sync if h == 0 else nc.scalar).dma_start(
                                out=t[:], in_=x[n, h * 128 : (h + 1) * 128]
                            )
                            xh.append(t)

                    # t-axis tree reduction on DVE: t 64 -> 32 -> 16 -> 8.
                    # The previous batch's reduce+q2 (and the 1-batch-old
                    # DVE score tail) are emitted between the two halves'
                    # trees so they never stall the in-order DVE queue.
                    r3 = []
                    for h in range(2):
                        a1 = xp.tile([128, HF // 2], FP, tag=f"a1{h}")
                        if n == 0:
                            # pair t,t+1 chunk-locally so each chunk's add
                            # can run as soon as its 1 MiB DMA lands
                            for c in range(2):
                                src = xh[h][
                                    :, c * 2048 : (c + 1) * 2048
                                ].rearrange(
                                    "p (t two v) -> p t two v", two=2, v=V
                                )
                                dst = a1[
                                    :, c * 1024 : (c + 1) * 1024
                                ].rearrange("p (t v) -> p t v", v=V)
                                nc.vector.tensor_add(
                                    dst, src[:, :, 0, :], src[:, :, 1, :]
                                )
                        else:
                            nc.vector.tensor_add(
                                a1[:], xh[h][:, : HF // 2], xh[h][:, HF // 2 :]
                            )
                        if h == 1:
                            flush_pending()
                        a2 = xp.tile([128, HF // 4], FP, tag=f"a2{h}")
                        nc.vector.tensor_add(
                            a2[:], a1[:, : HF // 4], a1[:, HF // 4 :]
                        )
                        a3 = xp.tile([128, HF // 8], FP, tag=f"a3{h}")
                        nc.vector.tensor_add(
                            a3[:], a2[:, : HF // 8], a2[:, HF // 8 :]
                        )
                        r3.append(a3)

                    # fused rest-of-mean + conv: accumulate c-halves into one
                    # psum group; psum free = (tl, v) partial t-sums
                    pq = psA.tile([INTER, 512], FP, tag="pq")
                    for h in range(2):
                        mm = nc.tensor.matmul(
                            pq[:],
                            wq_sb[:, h, :],
                            r3[h][:],
                            start=(h == 0),
                            stop=(h == 1),
                        )
                        if first_mm is None:
                            first_mm = mm
                            add_dep_helper(
                                mm.ins, d2.ins, sync=False,
                                reason="pe-wait-absorb order",
                            )
                    pending_q = (pq, n, 8)

            # drain: batch 7's reduce+q2, the 1-batch-old tail, then the
            # last pair fused (its flush_pending set pending_tail=(last
            # pair...) — but for the final pair we want the FUSED path, so
            # flush only the reduce/q2 by hand here).
            pqd, nd, ngroups = pending_q
            pending_q = None
            qtmp = small.tile([INTER, V], FP, tag="qtmp")
            nc.vector.reduce_sum(
                out=qtmp[:],
                in_=pqd[:, : ngroups * V].rearrange(
                    "o (t v) -> o v t", t=ngroups
                ),
                axis=mybir.AxisListType.X,
            )
            nc.scalar.activation(
                q2[:, V : 2 * V],
                qtmp[:],
                AF.Identity,
                bias=bq_sb[:],
                scale=1.0 / T,
            )
            if pending_tail is not None:
                emit_score_tail(*pending_tail)
                pending_tail = None
            emit_pb_chain(q2, fused_tail_p=nloc // 2 - 1)
    nc.finalize()
    return nc


_NC_CACHE: dict[int, bass.Bass] = {}


def _get_nc(nloc: int) -> bass.Bass:
    if nloc not in _NC_CACHE:
        _NC_CACHE[nloc] = build_nc(nloc)
    return _NC_CACHE[nloc]


def _make_in_maps(x, Wq, bq, key_prototypes, ncores):
    nloc = x.shape[0] // ncores
    wqt = np.ascontiguousarray(np.asarray(Wq, dtype=np.float32).T)
    kpc = np.ascontiguousarray(np.asarray(key_prototypes, dtype=np.float32))
    bqc = np.ascontiguousarray(
        np.asarray(bq, dtype=np.float32).reshape(INTER, 1)
    )
    xc = np.asarray(x, dtype=np.float32)
    return [
        {
            "x": np.ascontiguousarray(xc[i * nloc : (i + 1) * nloc]),
            "wqt": wqt,
            "kp": kpc,
            "bq": bqc,
        }
        for i in range(ncores)
    ]


def run(inputs, trace: bool = False):
    """Run on hardware; returns (full_output, BassKernelResults)."""
    from concourse.bass_utils import run_bass_kernel_spmd

    x = inputs["x"]
    nloc = x.shape[0] // NCORES
    nc = _get_nc(nloc)
    in_maps = _make_in_maps(
        x, inputs["Wq"], inputs["bq"], inputs["key_prototypes"], NCORES
    )
    res = run_bass_kernel_spmd(nc, in_maps, list(range(NCORES)), trace=trace)
    out = np.concatenate([r["out"] for r in res.results], axis=0)
    return out, res


def kernel(**inputs) -> np.ndarray:
    out, _ = run(inputs, trace=False)
    return out


# revision 24
# speedup vs baseline: 1.1578x; 1.1578x over previous
"""Trainium2 Bass kernel for DifferentiableSparseHypergraph (topk_masking).

Full computation per batch n:
  x_mean = x[n].mean(T)                      (C, V)
  q = Wq @ x_mean + bq                       (O=32, V)   [1x1 conv == matmul]
  q = q / max(||q||_2 over O, eps)
  H_raw = (q^T @ key_prototypes) / sqrt(O)   (V, M=128)
  topk10 -> softmax over the 10 vals -> scatter back; zeros elsewhere.

Kernel strategy (pure data-parallel over batch, 8 cores x 8 batches).
The kernel is DVE-bound (the t-mean tree must touch every element once
at ~1.1 ns/col; fp32 matmuls on the PE cost ~2.4 ns/col as LOW/HIGH
pairs, so offloading reduction to the PE loses), so everything is
arranged to keep the in-order DVE queue stall-free and its total work
minimal:
  * batches 0..6: one 4 MiB DMA per batch ([c, (half, t, v)]); t-mean
    via a 3-level DVE add tree per c-half (few BIG instructions -- DVE
    per-instruction overhead is ~100 ns, so many small ops lose ~20%
    throughput), a PSUM-accumulated N=512 fp32 matmul per half, and a
    strided-PSUM reduce over the 8 surviving t-groups.
  * each batch's reduce (which needs the PE matmul result) is emitted
    in the MIDDLE of the next batch's tree, so the in-order DVE queue
    never idles waiting for the PE (~1.2 us/batch otherwise).
  * batch 7 (LAST) streams in 1 MiB chunks with chunk-local 4-level
    trees and one N=256 matmul per c-half, so the compute tail after
    the final chunk lands is small instead of a whole batch's serial
    chain (~14 us measured on the coarse variant).
  * the score pipeline is split in two emission points so the DVE queue
    never stalls on cross-engine latency: the ACT/PE part (qsq, pb, pc,
    rn, e) is emitted right after the pair's q2; the DVE part (top-8
    threshold chain + masked-softmax) one batch later, by which time
    pb/e are long since ready. The last pair is emitted fused, with the
    DVE top-8 chain right after the pb matmul and pb before pc so the
    two engine chains overlap.
  * top-10 per row is index-free and SCALE-INVARIANT: the threshold
    t_k (10th largest) is found on the raw scores pb = q2^T kp; the
    normalization rn = (INTER*||q||^2)^-1/2 is applied inside the exp
    as a per-partition ACT scale. rn itself is exp(-0.5*ln(INTER*pc)):
    ln+exp+identity+copy+square all live in ONE activation table
    (natural_log_exp_and_others), and insert_act_table_loads is pinned
    to that table, so the kernel does zero mid-stream ACT table reloads
    (the stock greedy chooser flip-flops ln->natural_log /
    exp->exp_and_others at 1.28 us per swap, 2 per pair).
  * out = (pb >= t_k) * exp(rn*pb) / sum(...) == softmax over the top-10
    scattered back (softmax is shift/subset invariant).
"""

import types

import numpy as np

import concourse.bacc as bacc
import concourse.bass as bass
import concourse.mybir as mybir
import concourse.tile as tile

N, C, T, V = 64, 256, 64, 64
INTER = 32          # conv out channels
M = 128             # num hyperedges
TOPK = 10
NCORES = 8
FP = mybir.dt.float32
NEG_BIG = -1.0e30
TQ = 4              # t-quarters per c-half (1 MiB DMA chunks, last batch)
TC = T // TQ        # t's per chunk (16)
CHUNK_F = TC * V    # chunk free size (1024)
HF = T * V          # per-half free size (4096)

PINNED_ACT_TABLE = "natural_log_exp_and_others"


def _pinned_act_table_loads(self):
    """Replacement for Bacc.insert_act_table_loads on THIS instance only:
    mask every activation-function set except the pinned one so the
    fixpoint chooser can't flip-flop between tables (every function this
    kernel uses -- identity/copy/square/ln/exp -- lives in the pinned
    set). Set ids keep their positions so walrus still maps to the right
    runtime table."""
    import bass_rust as _bass_rust
    from concourse.hw_specs import get_activation_tables

    has_activation = any(
        isinstance(i, mybir.InstActivation)
        for b in self.main_func.blocks
        for i in b.instructions
    )
    if not has_activation:
        return
    tables = list(get_activation_tables(self.m.arch).items())
    assert any(name == PINNED_ACT_TABLE for name, _ in tables), tables
    pinned = [
        (name, (fns if name == PINNED_ACT_TABLE else set()))
        for name, fns in tables
    ]
    _bass_rust.insert_act_table_loads(self, pinned)


def build_nc(nloc: int) -> bass.Bass:
    """Build the per-core Bass program processing `nloc` batches."""
    assert nloc % 2 == 0
    # Bacc (not bare Bass): its compile()/finalize() pipeline splits
    # multi-semaphore waits into InstEventSemaphore pairs — walrus allows
    # at most one sync wait per regular instruction.
    nc = bacc.Bacc(target_bir_lowering=False, debug=False)
    nc.insert_act_table_loads = types.MethodType(_pinned_act_table_loads, nc)

    x = nc.dram_tensor("x", (nloc, C, T, V), FP, kind="ExternalInput")
    wqt = nc.dram_tensor("wqt", (C, INTER), FP, kind="ExternalInput")
    kp = nc.dram_tensor("kp", (INTER, M), FP, kind="ExternalInput")
    bq = nc.dram_tensor("bq", (INTER, 1), FP, kind="ExternalInput")
    out = nc.dram_tensor("out", (nloc, V, M), FP, kind="ExternalOutput")

    A = mybir.AluOpType
    AF = mybir.ActivationFunctionType
    from concourse.tile import add_dep_helper

    with tile.TileContext(nc) as tc:
        with (
            tc.tile_pool(name="consts", bufs=1) as consts,
            tc.tile_pool(name="xph", bufs=4) as xph,
            tc.tile_pool(name="xcp", bufs=8) as xcp,
            tc.tile_pool(name="xcs", bufs=2) as xcs,
            tc.tile_pool(name="xp", bufs=1) as xp,
            tc.tile_pool(name="small", bufs=2) as small,
            tc.tile_pool(name="psA", bufs=2, space="PSUM") as psA,
            tc.tile_pool(name="psB", bufs=2, space="PSUM") as psB,
            tc.tile_pool(name="psS", bufs=1, space="PSUM") as psS,
        ):
            # --- batch 0's x loads go FIRST (before the const DMAs) and in
            # 1 MiB t-range chunks, so the DVE tree starts as early as
            # possible.  Batch 0's first level pairs t with t+1 (each chunk
            # is self-contained); later batches pair t with t+32.
            xh0 = [
                xph.tile([128, HF], FP, tag=f"xh{h}", name=f"xh0_{h}")
                for h in range(2)
            ]
            for h in range(2):
                for c in range(2):
                    nc.sync.dma_start(
                        out=xh0[h][:, c * 2048 : (c + 1) * 2048],
                        in_=x[0, h * 128 : (h + 1) * 128,
                              c * (T // 2) : (c + 1) * (T // 2)],
                    )

            # --- replicated constants ---
            wq_sb = consts.tile([128, 2, INTER], FP)    # [c, c_half, o]
            nc.sync.dma_start(
                out=wq_sb[:], in_=wqt.rearrange("(h c) o -> c h o", h=2)
            )
            kp_sb = consts.tile([INTER, M], FP)
            nc.sync.dma_start(out=kp_sb[:], in_=kp[:])
            bq_sb = consts.tile([INTER, 1], FP)
            nc.sync.dma_start(out=bq_sb[:], in_=bq[:])
            ones_sb = consts.tile([INTER, 1], FP)
            nc.vector.memset(ones_sb[:], 1.0)

            # The fp32 self-loading matmul can carry at most ONE semaphore
            # wait (walrus S3_LW_STRUCT limit). Absorb the wq/kp DMA waits
            # with dummy 1x1 matmuls so the first real matmuls only wait on
            # their x-tile DMA.
            scr = psS.tile([1, 1], FP)
            d1 = nc.tensor.matmul(
                scr[:], wq_sb[:, 0, 0:1], wq_sb[:, 0, 0:1], start=True, stop=True
            )
            d2 = nc.tensor.matmul(
                scr[:], kp_sb[:, 0:1], kp_sb[:, 0:1], start=True, stop=True
            )
            add_dep_helper(d2.ins, d1.ins, sync=False, reason="pe-wait-absorb order")

            def emit_pb_chain(q2t, fused_tail_p=None):
                """ACT/PE part of a pair's scores: raw scores pb, row
                norms, e = exp(rn*pb). With fused_tail_p set (last pair),
                the DVE top-8 chain is emitted right after the pb matmul
                so both engine chains run concurrently in the tail."""
                qsq = small.tile([INTER, 2 * V], FP, tag="qsq")
                nc.scalar.activation(qsq[:], q2t[:], AF.Square)
                pb = psB.tile([2 * V, M], FP, tag="pb")
                nc.tensor.matmul(pb[:], q2t[:], kp_sb[:], start=True, stop=True)
                pc = psB.tile([2 * V, 1], FP, tag="pc")
                nc.tensor.matmul(
                    pc[:], qsq[:], ones_sb[:], start=True, stop=True
                )
                thr = emit_thresh(pb) if fused_tail_p is not None else None
                # rn = (INTER*||q||^2)^-0.5 = exp(-0.5*ln(INTER*pc))
                lns = small.tile([2 * V, 1], FP, tag="lns")
                nc.scalar.activation(lns[:], pc[:], AF.Ln, scale=float(INTER))
                rn = small.tile([2 * V, 1], FP, tag="rn")
                nc.scalar.activation(rn[:], lns[:], AF.Exp, scale=-0.5)
                e = small.tile([2 * V, M], FP, tag="e")
                nc.scalar.activation(e[:], pb[:], AF.Exp, scale=rn[:])
                if fused_tail_p is not None:
                    emit_softmax_tail(fused_tail_p, pb, e, thr)
                return pb, e

            def emit_thresh(pb):
                """DVE top-8 chain: t_k = 10th largest raw score per row
                (top8, knock them out, top8 again -> [:,1])."""
                top8a = small.tile([2 * V, 8], FP, tag="t8a")
                nc.vector.max(top8a[:], pb[:])
                work = small.tile([2 * V, M], FP, tag="work")
                nc.vector.match_replace(work[:], top8a[:], pb[:], NEG_BIG)
                top8b = small.tile([2 * V, 8], FP, tag="t8b")
                nc.vector.max(top8b[:], work[:])
                return top8b

            def emit_softmax_tail(p, pb, e, top8b):
                """masked softmax without scatter:
                me = (pb >= t_k) * e; out = me / sum(me)"""
                me = small.tile([2 * V, M], FP, tag="me")
                s = small.tile([2 * V, 1], FP, tag="s")
                nc.vector.scalar_tensor_tensor(
                    out=me[:],
                    in0=pb[:],
                    scalar=top8b[:, 1:2],
                    in1=e[:],
                    op0=A.is_ge,
                    op1=A.mult,
                    accum_out=s[:],
                )
                r = small.tile([2 * V, 1], FP, tag="r")
                nc.vector.reciprocal(r[:], s[:])
                ot = small.tile([2 * V, M], FP, tag="ot")
                nc.scalar.activation(ot[:], me[:], AF.Copy, scale=r[:])
                # out DMA on the ACT HWDGE ring so it never queues in front
                # of x-stream packets on the Sync ring's queue.
                nc.scalar.dma_start(
                    out=out[2 * p : 2 * p + 2].rearrange("b v m -> (b v) m"),
                    in_=ot[:],
                )

            def emit_score_tail(p, pb, e):
                emit_softmax_tail(p, pb, e, emit_thresh(pb))

            q2 = None
            first_mm = None
            pending_tail = None     # (p, pb, e) DVE tail, delayed 1 batch
            pending_q = None        # (pq_psum, n, groups), delayed 1/2 batch
            ready_pair = []         # pairs whose q2 completed at last flush

            def flush_pending():
                """Emit the previous batch's reduce+q2 (its PE matmul has
                had a whole tree's worth of time to finish), then the
                1-batch-old DVE score tail, then — if the flushed batch
                completed a pair — that pair's ACT/PE score chain."""
                nonlocal pending_q, pending_tail, q2
                if pending_q is None:
                    return
                pqd, nd, ngroups = pending_q
                pending_q = None
                ld = nd % 2
                qtmp = small.tile([INTER, V], FP, tag="qtmp")
                nc.vector.reduce_sum(
                    out=qtmp[:],
                    in_=pqd[:, : ngroups * V].rearrange(
                        "o (t v) -> o v t", t=ngroups
                    ),
                    axis=mybir.AxisListType.X,
                )
                # q = qsum/T + bq on the idle ACT engine
                if ld == 0:
                    q2 = small.tile([INTER, 2 * V], FP, tag="q2")
                nc.scalar.activation(
                    q2[:, ld * V : (ld + 1) * V],
                    qtmp[:],
                    AF.Identity,
                    bias=bq_sb[:],
                    scale=1.0 / T,
                )
                if pending_tail is not None:
                    emit_score_tail(*pending_tail)
                    pending_tail = None
                if ld == 1:
                    # q2 for pair nd//2 is complete; its ACT/PE score chain
                    # is emitted at the END of the current batch (after its
                    # conv matmuls) so pb/pc never sit ahead of the next
                    # conv matmul in the in-order PE queue (measured ~2 us
                    # TENSOR_REDUCE stall per pair batch otherwise).
                    ready_pair.append(nd // 2)

            for n in range(nloc):
                l = n % 2
                last = n == nloc - 1

                if last:
                    # --- fine-grained last batch: 1 MiB chunks, chunk-local
                    # 4-level trees, one N=256 matmul per c-half. (Same pool
                    # tag/shape as the coarse path so PSUM stays in budget.)
                    pqf = psA.tile([INTER, 512], FP, tag="pq")
                    for h in range(2):
                        cs7 = xcs.tile([128, TQ * V], FP, tag=f"cs7{h}")
                        for cq in range(TQ):
                            xc = xcp.tile([128, CHUNK_F], FP, tag="xc")
                            nc.sync.dma_start(
                                out=xc[:],
                                in_=x[n, h * 128 : (h + 1) * 128,
                                      cq * TC : (cq + 1) * TC],
                            )
                            c1 = xcs.tile([128, CHUNK_F // 2], FP, tag="c1")
                            nc.vector.tensor_add(
                                c1[:], xc[:, : CHUNK_F // 2],
                                xc[:, CHUNK_F // 2 :],
                            )
                            if h == 0 and cq == 1:
                                flush_pending()
                            c2 = xcs.tile([128, CHUNK_F // 4], FP, tag="c2")
                            nc.vector.tensor_add(
                                c2[:], c1[:, : CHUNK_F // 4],
                                c1[:, CHUNK_F // 4 :],
                            )
                            c3 = xcs.tile([128, CHUNK_F // 8], FP, tag="c3")
                            nc.vector.tensor_add(
                                c3[:], c2[:, : CHUNK_F // 8],
                                c2[:, CHUNK_F // 8 :],
                            )
                            nc.vector.tensor_add(
                                cs7[:, cq * V : (cq + 1) * V],
                                c3[:, :V], c3[:, V:],
                            )
                        nc.tensor.matmul(
                            pqf[:, : TQ * V], wq_sb[:, h, :], cs7[:],
                            start=(h == 0), stop=(h == 1),
                        )
                    pending_q = (pqf, n, TQ)
                else:
                    if n == 0:
                        xh = xh0
                    else:
                        xh = []
                        for h in range(2):
                            t = xph.tile([128, HF], FP, tag=f"xh{h}")
                            nc.sync.dma_start(
                                out=t[:], in_=x[n, h * 128 : (h + 1) * 128]
                            )
                            xh.append(t)

                    # t-axis tree reduction on DVE: t 64 -> 32 -> 16 -> 8.
                    # The previous batch's reduce+q2 (and the 1-batch-old
                    # DVE score tail) are emitted between the two halves'
                    # trees so they never stall the in-order DVE queue.
                    r3 = []
                    for h in range(2):
                        a1 = xp.tile([128, HF // 2], FP, tag=f"a1{h}")
                        if n == 0:
                            # pair t,t+1 chunk-locally so each chunk's add
                            # can run as soon as its 1 MiB DMA lands
                            for c in range(2):
                                src = xh[h][
                                    :, c * 2048 : (c + 1) * 2048
                                ].rearrange(
                                    "p (t two v) -> p t two v", two=2, v=V
                                )
                                dst = a1[
                                    :, c * 1024 : (c + 1) * 1024
                                ].rearrange("p (t v) -> p t v", v=V)
                                nc.vector.tensor_add(
                                    dst, src[:, :, 0, :], src[:, :, 1, :]
                                )
                        else:
                            nc.vector.tensor_add(
                                a1[:], xh[h][:, : HF // 2], xh[h][:, HF // 2 :]
                            )
                        if h == 1:
                            flush_pending()
                        a2 = xp.tile([128, HF // 4], FP, tag=f"a2{h}")
                        nc.vector.tensor_add(
                            a2[:], a1[:, : HF // 4], a1[:, HF // 4 :]
                        )
                        a3 = xp.tile([128, HF // 8], FP, tag=f"a3{h}")
                        nc.vector.tensor_add(
                            a3[:], a2[:, : HF // 8], a2[:, HF // 8 :]
                        )
                        r3.append(a3)

                    # fused rest-of-mean + conv: accumulate c-halves into one
                    # psum group; psum free = (tl, v) partial t-sums
                    pq = psA.tile([INTER, 512], FP, tag="pq")
                    for h in range(2):
                        mm = nc.tensor.matmul(
                            pq[:],
                            wq_sb[:, h, :],
                            r3[h][:],
                            start=(h == 0),
                            stop=(h == 1),
                        )
                        if first_mm is None:
                            first_mm = mm
                            add_dep_helper(
                                mm.ins, d2.ins, sync=False,
                                reason="pe-wait-absorb order",
                            )
                    pending_q = (pq, n, 8)

                while ready_pair:
                    pr = ready_pair.pop(0)
                    pb, e = emit_pb_chain(q2)
                    pending_tail = (pr, pb, e)

            # drain: batch 7's reduce+q2, the 1-batch-old tail, then the
            # last pair fused (its flush_pending set pending_tail=(last
            # pair...) — but for the final pair we want the FUSED path, so
            # flush only the reduce/q2 by hand here).
            pqd, nd, ngroups = pending_q
            pending_q = None
            qtmp = small.tile([INTER, V], FP, tag="qtmp")
            nc.vector.reduce_sum(
                out=qtmp[:],
                in_=pqd[:, : ngroups * V].rearrange(
                    "o (t v) -> o v t", t=ngroups
                ),
                axis=mybir.AxisListType.X,
            )
            nc.scalar.activation(
                q2[:, V : 2 * V],
                qtmp[:],
                AF.Identity,
                bias=bq_sb[:],
                scale=1.0 / T,
            )
            if pending_tail is not None:
                emit_score_tail(*pending_tail)
                pending_tail = None
            emit_pb_chain(q2, fused_tail_p=nloc // 2 - 1)
    nc.finalize()
    return nc


_NC_CACHE: dict[int, bass.Bass] = {}


def _get_nc(nloc: int) -> bass.Bass:
    if nloc not in _NC_CACHE:
        _NC_CACHE[nloc] = build_nc(nloc)
    return _NC_CACHE[nloc]


def _make_in_maps(x, Wq, bq, key_prototypes, ncores):
    nloc = x.shape[0] // ncores
    wqt = np.ascontiguousarray(np.asarray(Wq, dtype=np.float32).T)
    kpc = np.ascontiguousarray(np.asarray(key_prototypes, dtype=np.float32))
    bqc = np.ascontiguousarray(
        np.asarray(bq, dtype=np.float32).reshape(INTER, 1)
    )
    xc = np.asarray(x, dtype=np.float32)
    return [
        {
            "x": np.ascontiguousarray(xc[i * nloc : (i + 1) * nloc]),
            "wqt": wqt,
            "kp": kpc,
            "bq": bqc,
        }
        for i in range(ncores)
    ]


def run(inputs, trace: bool = False):
    """Run on hardware; returns (full_output, BassKernelResults)."""
    from concourse.bass_utils import run_bass_kernel_spmd

    x = inputs["x"]
    nloc = x.shape[0] // NCORES
    nc = _get_nc(nloc)
    in_maps = _make_in_maps(
        x, inputs["Wq"], inputs["bq"], inputs["key_prototypes"], NCORES
    )
    res = run_bass_kernel_spmd(nc, in_maps, list(range(NCORES)), trace=trace)
    out = np.concatenate([r["out"] for r in res.results], axis=0)
    return out, res


def kernel(**inputs) -> np.ndarray:
    out, _ = run(inputs, trace=False)
    return out
